# revision 10
# baseline (speedup 1.0000x reference)
"""Trainium2 Bass kernel for nn_CopyStack (copy-mechanism vocab scatter).

Computes, for full inputs:
    enc   = tanh(encoder_outputs @ W_proj + b_proj)          [B,S,H]
    score = decoder_outputs @ enc^T + input_bias             [B,T,S]
    probs = softmax(score, axis=-1)                          [B,T,S]
    out[b,t,v] = sum_{s: inputs[b,s]==v} probs[b,t,s]        [B,T,V]

Sharding: 8 cores; core c handles batch b=c//2, target rows
t in [128*(c%2), 128*(c%2)+128). Only W_proj/b_proj are replicated.

Device emits the scattered result in [V, TC] layout (row-contiguous
per vocab slot, which the HW indirect scatter supports natively); the
host unshard step transposes each shard into the final [B, T, V].

Scatter strategy: duplicate token ids are pre-combined with a matmul
against the S x S equality matrix C (every duplicate column carries
its group sum, so colliding DMA writes are identical), then a GPSIMD
indirect DMA scatters rows of probs2^T into the zero-filled [V, TC]
output. The 16.4 MB zero-fill runs on the sync/scalar/gpsimd DMA
queues concurrently with all compute, so the kernel's critical path
is the compute pipeline itself.

Precision: enc matmul + duplicate-combine run in fp16 (inputs pre-cast
on host), the scores matmul runs in fp32r (full fp32 precision at
fp16 PE throughput for wide moving tensors), softmax in fp32.
End-to-end rel err ~7e-3 vs the 2e-2 gate.
"""

import numpy as np

import concourse.bacc as bacc
import concourse.bass as bass
import concourse.tile as tile
from concourse import mybir
from concourse.bass import IndirectOffsetOnAxis
from concourse.bass_utils import run_bass_kernel_spmd
from concourse.masks import make_identity

F32 = mybir.dt.float32
F32R = mybir.dt.float32r
F16 = mybir.dt.float16
I32 = mybir.dt.int32

B, S, T, H, V = 4, 512, 256, 1024, 32000
TC = 128             # T-chunk per core
N_CORES = 8

KH = H // 128        # 8 hidden chunks
KS = S // 128        # 4 source chunks

ZCH = 4000           # zero-fill chunk width (f32 elems per partition)
NZCH = (V * TC) // (128 * ZCH)   # 8 chunks


def build_bass():
    nc = bacc.Bacc()

    e16 = nc.dram_tensor("e16", [S, H], F16, kind="ExternalInput")    # enc outs (fp16)
    d = nc.dram_tensor("d", [TC, H], F32, kind="ExternalInput")       # decoder chunk
    w16 = nc.dram_tensor("w16", [H, H], F16, kind="ExternalInput")    # W_proj (fp16)
    ids = nc.dram_tensor("ids", [S], I32, kind="ExternalInput")       # inputs[b]
    sbias = nc.dram_tensor("sbias", [S], F32, kind="ExternalInput")   # input_bias[b]
    bproj = nc.dram_tensor("bproj", [H], F32, kind="ExternalInput")   # b_proj
    out = nc.dram_tensor("out", [V, TC], F16, kind="ExternalOutput")

    with tile.TileContext(nc) as tc:
        with (
            tc.tile_pool(name="big", bufs=1) as big,
            tc.tile_pool(name="work", bufs=1) as work,
            tc.tile_pool(name="psum16", bufs=2, space="PSUM") as psum16,
            tc.tile_pool(name="psum32", bufs=2, space="PSUM") as psum32,
            tc.tile_pool(name="psum2", bufs=2, space="PSUM") as psum2,
            tc.tile_pool(name="psumacc", bufs=2, space="PSUM") as psumacc,
        ):
            # ---- input loads: sync queue carries the compute-critical
            # e16/d; scalar carries w16; gpsimd carries the row broadcasts,
            # so compute never waits behind zero-fill traffic ----
            d_sb = work.tile([128, H], F32, tag="d")
            nc.sync.dma_start(d_sb[:], d[:, :])
            ids_sb = work.tile([128, KS], I32, tag="ids")
            nc.sync.dma_start(ids_sb[:], ids[:].rearrange("(c p) -> p c", p=128))

            w_all = big.tile([128, KH * H], F16, tag="w_all")
            nc.scalar.dma_start(
                w_all[:].rearrange("p (c h) -> p c h", c=KH),
                w16[:, :].rearrange("(c p) h -> p c h", p=128))
            w_t = [w_all[:, k * H:(k + 1) * H] for k in range(KH)]

            ids_row_i = work.tile([128, S], I32, tag="ids_row_i")
            nc.gpsimd.dma_start(
                ids_row_i[:], ids[:].unsqueeze(0).to_broadcast([128, S]))
            bias_row = work.tile([128, S], F32, tag="bias_row")
            nc.gpsimd.dma_start(
                bias_row[:], sbias[:].unsqueeze(0).to_broadcast([128, S]))
            bproj_sb = work.tile([128, KH], F32, tag="bproj")
            nc.gpsimd.dma_start(bproj_sb[:], bproj[:].rearrange("(c p) -> p c", p=128))

            # ---- zero-fill out [V, TC] (overlaps all compute) ----
            zt = big.tile([128, ZCH], F16, tag="zt")
            nc.vector.memset(zt[:], 0.0)
            out_flat = out[:, :].rearrange("v t -> (v t)").rearrange(
                "(p f) -> p f", p=128)          # [128, 32000] flat view
            zq = [nc.sync, nc.scalar, nc.sync, nc.scalar,
                  nc.sync, nc.scalar, nc.gpsimd, nc.gpsimd]
            for j in range(NZCH):
                zq[j].dma_start(out_flat[:, j * ZCH:(j + 1) * ZCH], zt[:])

            # ---- identities ----
            ident16 = work.tile([128, 128], F16, tag="ident16")
            make_identity(nc, ident16[:])
            ident32 = work.tile([128, 128], F32, tag="ident32")
            make_identity(nc, ident32[:])

            # ---- E^T via DMA XBAR transpose (fp16): eT[k] = [128(h), S] ----
            eT = []
            for k in range(KH):
                t_ = big.tile([128, S], F16, tag=f"eT{k}")
                q = nc.sync if k % 2 == 0 else nc.scalar
                q.dma_start_transpose(t_[:], e16[:, k * 128:(k + 1) * 128])
                eT.append(t_)

            # ---- D^T early (fp32 PE transpose; PE is idle pre-enc) ----
            dT = []
            for k in range(KH):
                pt = psum32.tile([128, 128], F32, tag="tp32")
                nc.tensor.transpose(
                    out=pt[:], in_=d_sb[:, k * 128:(k + 1) * 128], identity=ident32[:],
                )
                t_ = work.tile([128, 128], F32R, tag=f"dT{k}")
                nc.vector.tensor_copy(t_[:], pt[:])
                dT.append(t_)

            # ---- encT[m] = tanh(W^T @ E^T + b)  -> [128(h'), S] f32 ----
            encT = []
            for m in range(KH):
                pm = psumacc.tile([128, S], F32, tag="mm")
                for k in range(KH):
                    nc.tensor.matmul(
                        pm[:], lhsT=w_t[k][:, m * 128:(m + 1) * 128], rhs=eT[k][:],
                        start=(k == 0), stop=(k == KH - 1),
                    )
                et = big.tile([128, S], F32R, tag=f"encT{m}")
                nc.scalar.activation(
                    et[:], pm[:], mybir.ActivationFunctionType.Tanh,
                    bias=bproj_sb[:, m:m + 1], scale=1.0,
                )
                encT.append(et)

            # ---- scores[t,s] = sum_h' dT[h',t] * encT[h',s]  (fp32r) ----
            ps = psumacc.tile([128, S], F32, tag="mm")
            for k in range(KH):
                nc.tensor.matmul(
                    ps[:], lhsT=dT[k][:], rhs=encT[k][:],
                    start=(k == 0), stop=(k == KH - 1),
                )

            scoresb = work.tile([128, S], F32, tag="scoresb")
            nc.vector.tensor_tensor(
                out=scoresb[:], in0=ps[:], in1=bias_row[:], op=mybir.AluOpType.add,
            )

            # ---- softmax over s (fp32), probs emitted as fp16 ----
            rmax = work.tile([128, 1], F32, tag="rmax")
            nc.vector.reduce_max(rmax[:], scoresb[:], axis=mybir.AxisListType.X)
            nrmax = work.tile([128, 1], F32, tag="nrmax")
            nc.vector.tensor_scalar_mul(nrmax[:], rmax[:], -1.0)
            ex = work.tile([128, S], F32, tag="ex")
            rsum = work.tile([128, 1], F32, tag="rsum")
            nc.scalar.activation(
                ex[:], scoresb[:], mybir.ActivationFunctionType.Exp,
                bias=nrmax[:, :1], scale=1.0, accum_out=rsum[:, :1],
            )
            rinv = work.tile([128, 1], F32, tag="rinv")
            nc.vector.reciprocal(rinv[:], rsum[:])
            probs16 = work.tile([128, S], F16, tag="probs16")
            nc.vector.tensor_scalar_mul(probs16[:], ex[:], rinv[:, :1])

            # ---- C_k[p, f] = (ids[128k+p] == ids[f])  (fp16) ----
            ids_f = work.tile([128, KS], F32, tag="ids_f")
            nc.vector.tensor_copy(ids_f[:], ids_sb[:])
            ids_row_f = work.tile([128, S], F32, tag="ids_row_f")
            nc.gpsimd.tensor_copy(ids_row_f[:], ids_row_i[:])
            C = []
            for k in range(KS):
                ck = work.tile([128, S], F16, tag=f"C{k}")
                nc.vector.tensor_tensor(
                    out=ck[:],
                    in0=ids_f[:, k:k + 1].to_broadcast([128, S]),
                    in1=ids_row_f[:],
                    op=mybir.AluOpType.is_equal,
                )
                C.append(ck)

            # ---- probsT (fp16 PE transpose) ----
            pT = []
            for k in range(KS):
                pt = psum16.tile([128, 128], F16, tag="tp16")
                nc.tensor.transpose(
                    out=pt[:], in_=probs16[:, k * 128:(k + 1) * 128],
                    identity=ident16[:],
                )
                t_ = work.tile([128, 128], F16, tag=f"pT{k}")
                nc.vector.tensor_copy(t_[:], pt[:])
                pT.append(t_)

            # ---- p2T[j] = sum_k C_k[:, j]^T @ probsT_k (group sums, fp16 mm)
            #      then indirect row scatter: out[ids[s], :] = p2T rows ----
            for j in range(KS):
                pj = psum2.tile([128, 128], F32, tag="mm2")
                for k in range(KS):
                    nc.tensor.matmul(
                        pj[:], lhsT=C[k][:, j * 128:(j + 1) * 128], rhs=pT[k][:],
                        start=(k == 0), stop=(k == KS - 1),
                    )
                p2 = work.tile([128, 128], F16, tag=f"p2T{j}")
                if j % 2 == 0:
                    nc.vector.tensor_copy(p2[:], pj[:])
                else:
                    nc.scalar.copy(p2[:], pj[:])
                nc.gpsimd.indirect_dma_start(
                    out=out[:, :],
                    out_offset=IndirectOffsetOnAxis(ap=ids_sb[:, j:j + 1], axis=0),
                    in_=p2[:],
                    in_offset=None,
                )

    nc.finalize()
    return nc


_NC_CACHE = None


def _get_nc():
    global _NC_CACHE
    if _NC_CACHE is None:
        _NC_CACHE = build_bass()
    return _NC_CACHE


def kernel(**inputs: np.ndarray) -> np.ndarray:
    E = np.asarray(inputs["encoder_outputs"], dtype=np.float32)
    D = np.asarray(inputs["decoder_outputs"], dtype=np.float32)
    ids = np.ascontiguousarray(np.asarray(inputs["inputs"]).astype(np.int32))
    ib = np.ascontiguousarray(np.asarray(inputs["input_bias"], dtype=np.float32))
    W = np.asarray(inputs["W_proj"], dtype=np.float32)
    bp = np.ascontiguousarray(np.asarray(inputs["b_proj"], dtype=np.float32))

    E16 = np.ascontiguousarray(E.astype(np.float16))
    W16 = np.ascontiguousarray(W.astype(np.float16))

    nc = _get_nc()
    in_maps = []
    for c in range(N_CORES):
        b, th = c // 2, c % 2
        in_maps.append({
            "e16": E16[b],
            "d": np.ascontiguousarray(D[b, th * TC:(th + 1) * TC]),
            "w16": W16,
            "ids": ids[b],
            "sbias": ib[b],
            "bproj": bp,
        })
    res = run_bass_kernel_spmd(nc, in_maps, core_ids=list(range(N_CORES)))
    out = np.empty((B, T, V), dtype=np.float32)
    for c in range(N_CORES):
        b, th = c // 2, c % 2
        shard = res.results[c]["out"]          # [V, TC]
        out[b, th * TC:(th + 1) * TC, :] = shard.T
    return out


if __name__ == "__main__":
    nc = build_bass()
    print("built ok")


# revision 12
# speedup vs baseline: 1.3065x; 1.3065x over previous
"""Trainium2 Bass kernel for nn_CopyStack (copy-mechanism vocab scatter).

Computes, for full inputs:
    enc   = tanh(encoder_outputs @ W_proj + b_proj)          [B,S,H]
    score = decoder_outputs @ enc^T + input_bias             [B,T,S]
    probs = softmax(score, axis=-1)                          [B,T,S]
    out[b,t,v] = sum_{s: inputs[b,s]==v} probs[b,t,s]        [B,T,V]

Sharding: 8 cores; core c handles batch b=c//2, target rows
t in [128*(c%2), 128*(c%2)+128). Only W_proj/b_proj are replicated.

Device emits the scattered result in [V, TC] layout (row-contiguous
per vocab slot, which the HW indirect scatter supports natively); the
host unshard step transposes each shard into the final [B, T, V].

Scatter strategy: duplicate token ids are pre-combined with a matmul
against the S x S equality matrix C (every duplicate column carries
its group sum, so colliding DMA writes are identical), then a GPSIMD
indirect DMA scatters rows of probs2^T into the zero-filled [V, TC]
output. The 16.4 MB zero-fill runs on the sync/scalar/gpsimd DMA
queues concurrently with all compute, so the kernel's critical path
is the compute pipeline itself.

Precision: enc matmul + duplicate-combine run in fp16 (inputs pre-cast
on host), the scores matmul runs in fp32r (full fp32 precision at
fp16 PE throughput for wide moving tensors), softmax in fp32.
End-to-end rel err ~7e-3 vs the 2e-2 gate.
"""

import numpy as np

import concourse.bacc as bacc
import concourse.bass as bass
import concourse.tile as tile
from concourse import mybir
from concourse.bass import IndirectOffsetOnAxis
from concourse.bass_utils import run_bass_kernel_spmd
from concourse.masks import make_identity

F32 = mybir.dt.float32
F32R = mybir.dt.float32r
F16 = mybir.dt.float16
I32 = mybir.dt.int32

B, S, T, H, V = 4, 512, 256, 1024, 32000
TC = 128             # T-chunk per core
N_CORES = 8

KH = H // 128        # 8 hidden chunks
KS = S // 128        # 4 source chunks

ZCH = 4000           # zero-fill chunk width (f32 elems per partition)
NZCH = (V * TC) // (128 * ZCH)   # 8 chunks


def build_bass():
    nc = bacc.Bacc()

    e16t = nc.dram_tensor("e16t", [H, S], F16, kind="ExternalInput")  # enc outs^T (fp16)
    d32t = nc.dram_tensor("d32t", [H, TC], F32R, kind="ExternalInput")  # decoder chunk^T
    w16 = nc.dram_tensor("w16", [H, H], F16, kind="ExternalInput")    # W_proj (fp16)
    ids = nc.dram_tensor("ids", [S], I32, kind="ExternalInput")       # inputs[b]
    sbias = nc.dram_tensor("sbias", [S], F32, kind="ExternalInput")   # input_bias[b]
    bproj = nc.dram_tensor("bproj", [H], F32, kind="ExternalInput")   # b_proj
    out = nc.dram_tensor("out", [V, TC], F16, kind="ExternalOutput")

    with tile.TileContext(nc) as tc:
        with (
            tc.tile_pool(name="big", bufs=1) as big,
            tc.tile_pool(name="work", bufs=1) as work,
            tc.tile_pool(name="psum16", bufs=2, space="PSUM") as psum16,
            tc.tile_pool(name="psum2", bufs=2, space="PSUM") as psum2,
            tc.tile_pool(name="psumacc", bufs=2, space="PSUM") as psumacc,
        ):
            # ---- input loads: sync queue carries the compute-critical
            # e16/d; scalar carries w16; gpsimd carries the row broadcasts,
            # so compute never waits behind zero-fill traffic ----
            eT_all = big.tile([128, KH * S], F16, tag="eT_all")
            nc.sync.dma_start(
                eT_all[:].rearrange("p (k s) -> p k s", k=KH),
                e16t[:, :].rearrange("(k p) s -> p k s", p=128))
            eT = [eT_all[:, k * S:(k + 1) * S] for k in range(KH)]
            dT_all = work.tile([128, KH * TC], F32R, tag="dT_all")
            nc.sync.dma_start(
                dT_all[:].rearrange("p (k t) -> p k t", k=KH),
                d32t[:, :].rearrange("(k p) t -> p k t", p=128))
            dT = [dT_all[:, k * TC:(k + 1) * TC] for k in range(KH)]
            ids_sb = work.tile([128, KS], I32, tag="ids")
            nc.sync.dma_start(ids_sb[:], ids[:].rearrange("(c p) -> p c", p=128))

            w_all = big.tile([128, KH * H], F16, tag="w_all")
            nc.scalar.dma_start(
                w_all[:].rearrange("p (c h) -> p c h", c=KH),
                w16[:, :].rearrange("(c p) h -> p c h", p=128))
            w_t = [w_all[:, k * H:(k + 1) * H] for k in range(KH)]

            ids_row_i = work.tile([128, S], I32, tag="ids_row_i")
            nc.gpsimd.dma_start(
                ids_row_i[:], ids[:].unsqueeze(0).to_broadcast([128, S]))
            bias_row = work.tile([128, S], F32, tag="bias_row")
            nc.gpsimd.dma_start(
                bias_row[:], sbias[:].unsqueeze(0).to_broadcast([128, S]))
            bproj_sb = work.tile([128, KH], F32, tag="bproj")
            nc.gpsimd.dma_start(bproj_sb[:], bproj[:].rearrange("(c p) -> p c", p=128))

            # ---- zero-fill out [V, TC] (overlaps all compute) ----
            zt = big.tile([128, ZCH], F16, tag="zt")
            nc.vector.memset(zt[:], 0.0)
            out_flat = out[:, :].rearrange("v t -> (v t)").rearrange(
                "(p f) -> p f", p=128)          # [128, 32000] flat view
            zq = [nc.sync, nc.scalar, nc.sync, nc.scalar,
                  nc.sync, nc.scalar, nc.gpsimd, nc.gpsimd]
            for j in range(NZCH):
                zq[j].dma_start(out_flat[:, j * ZCH:(j + 1) * ZCH], zt[:])

            # ---- identities ----
            ident16 = work.tile([128, 128], F16, tag="ident16")
            make_identity(nc, ident16[:])

            # ---- encT[m] = tanh(W^T @ E^T + b)  -> [128(h'), S] f32 ----
            encT = []
            for m in range(KH):
                pm = psumacc.tile([128, S], F32, tag="mm")
                for k in range(KH):
                    nc.tensor.matmul(
                        pm[:], lhsT=w_t[k][:, m * 128:(m + 1) * 128], rhs=eT[k][:],
                        start=(k == 0), stop=(k == KH - 1),
                    )
                et = big.tile([128, S], F32R, tag=f"encT{m}")
                nc.scalar.activation(
                    et[:], pm[:], mybir.ActivationFunctionType.Tanh,
                    bias=bproj_sb[:, m:m + 1], scale=1.0,
                )
                encT.append(et)

            # ---- scores[t,s] = sum_h' dT[h',t] * encT[h',s]  (fp32r) ----
            ps = psumacc.tile([128, S], F32, tag="mm")
            for k in range(KH):
                nc.tensor.matmul(
                    ps[:], lhsT=dT[k][:], rhs=encT[k][:],
                    start=(k == 0), stop=(k == KH - 1),
                )

            # ---- softmax over s (fp32), probs emitted as fp16 ----
            scoresb = work.tile([128, S], F32, tag="scoresb")
            nc.vector.tensor_tensor(
                out=scoresb[:], in0=ps[:], in1=bias_row[:], op=mybir.AluOpType.add,
            )
            rmax = work.tile([128, 1], F32, tag="rmax")
            nc.vector.reduce_max(rmax[:], scoresb[:], axis=mybir.AxisListType.X)
            nrmax = work.tile([128, 1], F32, tag="nrmax")
            nc.vector.tensor_scalar_mul(nrmax[:], rmax[:], -1.0)
            ex = work.tile([128, S], F32, tag="ex")
            rsum = work.tile([128, 1], F32, tag="rsum")
            nc.scalar.activation(
                ex[:], scoresb[:], mybir.ActivationFunctionType.Exp,
                bias=nrmax[:, :1], scale=1.0, accum_out=rsum[:, :1],
            )
            rinv = work.tile([128, 1], F32, tag="rinv")
            nc.vector.reciprocal(rinv[:], rsum[:])
            probs16 = work.tile([128, S], F16, tag="probs16")
            nc.vector.tensor_scalar_mul(probs16[:], ex[:], rinv[:, :1])

            # ---- C_k[p, f] = (ids[128k+p] == ids[f])  (fp16) ----
            ids_f = work.tile([128, KS], F32, tag="ids_f")
            nc.vector.tensor_copy(ids_f[:], ids_sb[:])
            ids_row_f = work.tile([128, S], F32, tag="ids_row_f")
            nc.gpsimd.tensor_copy(ids_row_f[:], ids_row_i[:])
            C = []
            for k in range(KS):
                ck = work.tile([128, S], F16, tag=f"C{k}")
                nc.vector.tensor_tensor(
                    out=ck[:],
                    in0=ids_f[:, k:k + 1].to_broadcast([128, S]),
                    in1=ids_row_f[:],
                    op=mybir.AluOpType.is_equal,
                )
                C.append(ck)

            # ---- probsT (fp16 PE transpose) ----
            pT = []
            for k in range(KS):
                pt = psum16.tile([128, 128], F16, tag="tp16")
                nc.tensor.transpose(
                    out=pt[:], in_=probs16[:, k * 128:(k + 1) * 128],
                    identity=ident16[:],
                )
                t_ = work.tile([128, 128], F16, tag=f"pT{k}")
                nc.vector.tensor_copy(t_[:], pt[:])
                pT.append(t_)

            # ---- p2T[j] = sum_k C_k[:, j]^T @ probsT_k (group sums, fp16 mm)
            #      then indirect row scatter: out[ids[s], :] = p2T rows ----
            for j in range(KS):
                pj = psum2.tile([128, 128], F32, tag="mm2")
                for k in range(KS):
                    nc.tensor.matmul(
                        pj[:], lhsT=C[k][:, j * 128:(j + 1) * 128], rhs=pT[k][:],
                        start=(k == 0), stop=(k == KS - 1),
                    )
                p2 = work.tile([128, 128], F16, tag=f"p2T{j}")
                if j % 2 == 0:
                    nc.vector.tensor_copy(p2[:], pj[:])
                else:
                    nc.scalar.copy(p2[:], pj[:])
                nc.gpsimd.indirect_dma_start(
                    out=out[:, :],
                    out_offset=IndirectOffsetOnAxis(ap=ids_sb[:, j:j + 1], axis=0),
                    in_=p2[:],
                    in_offset=None,
                )

    nc.finalize()
    return nc


_NC_CACHE = None


def _get_nc():
    global _NC_CACHE
    if _NC_CACHE is None:
        _NC_CACHE = build_bass()
    return _NC_CACHE


def kernel(**inputs: np.ndarray) -> np.ndarray:
    E = np.asarray(inputs["encoder_outputs"], dtype=np.float32)
    D = np.asarray(inputs["decoder_outputs"], dtype=np.float32)
    ids = np.ascontiguousarray(np.asarray(inputs["inputs"]).astype(np.int32))
    ib = np.ascontiguousarray(np.asarray(inputs["input_bias"], dtype=np.float32))
    W = np.asarray(inputs["W_proj"], dtype=np.float32)
    bp = np.ascontiguousarray(np.asarray(inputs["b_proj"], dtype=np.float32))

    E16T = np.ascontiguousarray(E.astype(np.float16).transpose(0, 2, 1))
    W16 = np.ascontiguousarray(W.astype(np.float16))
    DT = np.ascontiguousarray(D.transpose(0, 2, 1))

    nc = _get_nc()
    in_maps = []
    for c in range(N_CORES):
        b, th = c // 2, c % 2
        in_maps.append({
            "e16t": E16T[b],
            "d32t": np.ascontiguousarray(DT[b, :, th * TC:(th + 1) * TC]),
            "w16": W16,
            "ids": ids[b],
            "sbias": ib[b],
            "bproj": bp,
        })
    res = run_bass_kernel_spmd(nc, in_maps, core_ids=list(range(N_CORES)))
    out = np.empty((B, T, V), dtype=np.float32)
    for c in range(N_CORES):
        b, th = c // 2, c % 2
        shard = res.results[c]["out"]          # [V, TC]
        out[b, th * TC:(th + 1) * TC, :] = shard.T
    return out


if __name__ == "__main__":
    nc = build_bass()
    print("built ok")


# revision 13
# speedup vs baseline: 1.3626x; 1.0429x over previous
"""Trainium2 Bass kernel for nn_CopyStack (copy-mechanism vocab scatter).

Computes, for full inputs:
    enc   = tanh(encoder_outputs @ W_proj + b_proj)          [B,S,H]
    score = decoder_outputs @ enc^T + input_bias             [B,T,S]
    probs = softmax(score, axis=-1)                          [B,T,S]
    out[b,t,v] = sum_{s: inputs[b,s]==v} probs[b,t,s]        [B,T,V]

Sharding: 8 cores; core c handles batch b=c//2, target rows
t in [128*(c%2), 128*(c%2)+128). Only W_proj/b_proj are replicated.

Device emits the scattered result in [V, TC] layout (row-contiguous
per vocab slot, which the HW indirect scatter supports natively); the
host unshard step transposes each shard into the final [B, T, V].

Scatter strategy: duplicate token ids are pre-combined with a matmul
against the S x S equality matrix C (every duplicate column carries
its group sum, so colliding DMA writes are identical), then a GPSIMD
indirect DMA scatters rows of probs2^T into the zero-filled [V, TC]
output. The 16.4 MB zero-fill runs on the sync/scalar/gpsimd DMA
queues concurrently with all compute, so the kernel's critical path
is the compute pipeline itself.

Precision: enc matmul + duplicate-combine run in fp16 (inputs pre-cast
on host), the scores matmul runs in fp32r (full fp32 precision at
fp16 PE throughput for wide moving tensors), softmax in fp32.
End-to-end rel err ~7e-3 vs the 2e-2 gate.
"""

import numpy as np

import concourse.bacc as bacc
import concourse.bass as bass
import concourse.tile as tile
from concourse import mybir
from concourse.bass import IndirectOffsetOnAxis
from concourse.bass_utils import run_bass_kernel_spmd
from concourse.masks import make_identity

F32 = mybir.dt.float32
F32R = mybir.dt.float32r
F16 = mybir.dt.float16
I32 = mybir.dt.int32

B, S, T, H, V = 4, 512, 256, 1024, 32000
TC = 128             # T-chunk per core
N_CORES = 8

KH = H // 128        # 8 hidden chunks
KS = S // 128        # 4 source chunks

ZCH = 4000           # zero-fill chunk width (f32 elems per partition)
NZCH = (V * TC) // (128 * ZCH)   # 8 chunks


def build_bass():
    nc = bacc.Bacc()

    # pre-tiled inputs: [128, k*free] with tile[p, k*F+f] = src[128k+p, f]
    e16t = nc.dram_tensor("e16t", [128, KH * S], F16, kind="ExternalInput")
    d32t = nc.dram_tensor("d32t", [128, KH * TC], F32R, kind="ExternalInput")
    w16 = nc.dram_tensor("w16", [128, KH * H], F16, kind="ExternalInput")
    ids = nc.dram_tensor("ids", [S], I32, kind="ExternalInput")       # inputs[b]
    sbias = nc.dram_tensor("sbias", [S], F32, kind="ExternalInput")   # input_bias[b]
    bproj = nc.dram_tensor("bproj", [H], F32, kind="ExternalInput")   # b_proj
    out = nc.dram_tensor("out", [V, TC], F16, kind="ExternalOutput")

    with tile.TileContext(nc) as tc:
        with (
            tc.tile_pool(name="big", bufs=1) as big,
            tc.tile_pool(name="work", bufs=1) as work,
            tc.tile_pool(name="psum16", bufs=2, space="PSUM") as psum16,
            tc.tile_pool(name="psum2", bufs=2, space="PSUM") as psum2,
            tc.tile_pool(name="psumacc", bufs=2, space="PSUM") as psumacc,
        ):
            # ---- input loads: sync queue carries the compute-critical
            # e16/d; scalar carries w16; gpsimd carries the row broadcasts,
            # so compute never waits behind zero-fill traffic ----
            eT_all = big.tile([128, KH * S], F16, tag="eT_all")
            nc.sync.dma_start(eT_all[:], e16t[:, :])
            eT = [eT_all[:, k * S:(k + 1) * S] for k in range(KH)]
            dT_all = work.tile([128, KH * TC], F32R, tag="dT_all")
            nc.sync.dma_start(dT_all[:], d32t[:, :])
            dT = [dT_all[:, k * TC:(k + 1) * TC] for k in range(KH)]
            ids_sb = work.tile([128, KS], I32, tag="ids")
            nc.sync.dma_start(ids_sb[:], ids[:].rearrange("(c p) -> p c", p=128))

            w_all = big.tile([128, KH * H], F16, tag="w_all")
            nc.scalar.dma_start(w_all[:], w16[:, :])
            w_t = [w_all[:, k * H:(k + 1) * H] for k in range(KH)]

            ids_row_i = work.tile([128, S], I32, tag="ids_row_i")
            nc.gpsimd.dma_start(
                ids_row_i[:], ids[:].unsqueeze(0).to_broadcast([128, S]))
            bias_row = work.tile([128, S], F32, tag="bias_row")
            nc.gpsimd.dma_start(
                bias_row[:], sbias[:].unsqueeze(0).to_broadcast([128, S]))
            bproj_sb = work.tile([128, KH], F32, tag="bproj")
            nc.gpsimd.dma_start(bproj_sb[:], bproj[:].rearrange("(c p) -> p c", p=128))

            # ---- zero-fill out [V, TC] (overlaps all compute) ----
            zt = big.tile([128, ZCH], F16, tag="zt")
            nc.vector.memset(zt[:], 0.0)
            out_flat = out[:, :].rearrange("v t -> (v t)").rearrange(
                "(p f) -> p f", p=128)          # [128, 32000] flat view
            zq = [nc.sync, nc.scalar, nc.sync, nc.scalar,
                  nc.sync, nc.scalar, nc.gpsimd, nc.gpsimd]
            for j in range(NZCH):
                zq[j].dma_start(out_flat[:, j * ZCH:(j + 1) * ZCH], zt[:])

            # ---- identities ----
            ident16 = work.tile([128, 128], F16, tag="ident16")
            make_identity(nc, ident16[:])

            # ---- encT[m] = tanh(W^T @ E^T + b)  -> [128(h'), S] f32 ----
            encT = []
            for m in range(KH):
                pm = psumacc.tile([128, S], F32, tag="mm")
                for k in range(KH):
                    nc.tensor.matmul(
                        pm[:], lhsT=w_t[k][:, m * 128:(m + 1) * 128], rhs=eT[k][:],
                        start=(k == 0), stop=(k == KH - 1),
                    )
                et = big.tile([128, S], F32R, tag=f"encT{m}")
                nc.scalar.activation(
                    et[:], pm[:], mybir.ActivationFunctionType.Tanh,
                    bias=bproj_sb[:, m:m + 1], scale=1.0,
                )
                encT.append(et)

            # ---- scores[t,s] = sum_h' dT[h',t] * encT[h',s]  (fp32r) ----
            ps = psumacc.tile([128, S], F32, tag="mm")
            for k in range(KH):
                nc.tensor.matmul(
                    ps[:], lhsT=dT[k][:], rhs=encT[k][:],
                    start=(k == 0), stop=(k == KH - 1),
                )

            # ---- softmax over s (fp32), probs emitted as fp16 ----
            scoresb = work.tile([128, S], F32, tag="scoresb")
            nc.vector.tensor_tensor(
                out=scoresb[:], in0=ps[:], in1=bias_row[:], op=mybir.AluOpType.add,
            )
            rmax = work.tile([128, 1], F32, tag="rmax")
            nc.vector.reduce_max(rmax[:], scoresb[:], axis=mybir.AxisListType.X)
            nrmax = work.tile([128, 1], F32, tag="nrmax")
            nc.vector.tensor_scalar_mul(nrmax[:], rmax[:], -1.0)
            ex = work.tile([128, S], F32, tag="ex")
            rsum = work.tile([128, 1], F32, tag="rsum")
            nc.scalar.activation(
                ex[:], scoresb[:], mybir.ActivationFunctionType.Exp,
                bias=nrmax[:, :1], scale=1.0, accum_out=rsum[:, :1],
            )
            rinv = work.tile([128, 1], F32, tag="rinv")
            nc.vector.reciprocal(rinv[:], rsum[:])
            probs16 = work.tile([128, S], F16, tag="probs16")
            nc.vector.tensor_scalar_mul(probs16[:], ex[:], rinv[:, :1])

            # ---- C_k[p, f] = (ids[128k+p] == ids[f])  (fp16) ----
            ids_f = work.tile([128, KS], F32, tag="ids_f")
            nc.vector.tensor_copy(ids_f[:], ids_sb[:])
            ids_row_f = work.tile([128, S], F32, tag="ids_row_f")
            nc.gpsimd.tensor_copy(ids_row_f[:], ids_row_i[:])
            C = []
            for k in range(KS):
                ck = work.tile([128, S], F16, tag=f"C{k}")
                nc.vector.tensor_tensor(
                    out=ck[:],
                    in0=ids_f[:, k:k + 1].to_broadcast([128, S]),
                    in1=ids_row_f[:],
                    op=mybir.AluOpType.is_equal,
                )
                C.append(ck)

            # ---- probsT (fp16 PE transpose) ----
            pT = []
            for k in range(KS):
                pt = psum16.tile([128, 128], F16, tag="tp16")
                nc.tensor.transpose(
                    out=pt[:], in_=probs16[:, k * 128:(k + 1) * 128],
                    identity=ident16[:],
                )
                t_ = work.tile([128, 128], F16, tag=f"pT{k}")
                nc.vector.tensor_copy(t_[:], pt[:])
                pT.append(t_)

            # ---- p2T[j] = sum_k C_k[:, j]^T @ probsT_k (group sums, fp16 mm)
            #      then indirect row scatter: out[ids[s], :] = p2T rows ----
            for j in range(KS):
                pj = psum2.tile([128, 128], F32, tag="mm2")
                for k in range(KS):
                    nc.tensor.matmul(
                        pj[:], lhsT=C[k][:, j * 128:(j + 1) * 128], rhs=pT[k][:],
                        start=(k == 0), stop=(k == KS - 1),
                    )
                p2 = work.tile([128, 128], F16, tag=f"p2T{j}")
                if j % 2 == 0:
                    nc.vector.tensor_copy(p2[:], pj[:])
                else:
                    nc.scalar.copy(p2[:], pj[:])
                nc.gpsimd.indirect_dma_start(
                    out=out[:, :],
                    out_offset=IndirectOffsetOnAxis(ap=ids_sb[:, j:j + 1], axis=0),
                    in_=p2[:],
                    in_offset=None,
                )

    nc.finalize()
    return nc


_NC_CACHE = None


def _get_nc():
    global _NC_CACHE
    if _NC_CACHE is None:
        _NC_CACHE = build_bass()
    return _NC_CACHE


def kernel(**inputs: np.ndarray) -> np.ndarray:
    E = np.asarray(inputs["encoder_outputs"], dtype=np.float32)
    D = np.asarray(inputs["decoder_outputs"], dtype=np.float32)
    ids = np.ascontiguousarray(np.asarray(inputs["inputs"]).astype(np.int32))
    ib = np.ascontiguousarray(np.asarray(inputs["input_bias"], dtype=np.float32))
    W = np.asarray(inputs["W_proj"], dtype=np.float32)
    bp = np.ascontiguousarray(np.asarray(inputs["b_proj"], dtype=np.float32))

    # tile[p, k*F+f] = src[128k+p, f] layouts, fully contiguous on device
    E16T = E.astype(np.float16).transpose(0, 2, 1).reshape(B, 8, 128, 512)\
        .transpose(0, 2, 1, 3).reshape(B, 128, 8 * 512)
    W16 = np.ascontiguousarray(
        W.astype(np.float16).reshape(8, 128, 1024).transpose(1, 0, 2)
        .reshape(128, 8 * 1024))
    DT = D.transpose(0, 2, 1).reshape(B, 8, 128, 256)

    nc = _get_nc()
    in_maps = []
    for c in range(N_CORES):
        b, th = c // 2, c % 2
        in_maps.append({
            "e16t": np.ascontiguousarray(E16T[b]),
            "d32t": np.ascontiguousarray(
                DT[b, :, :, th * TC:(th + 1) * TC].transpose(1, 0, 2)
                .reshape(128, 8 * TC)),
            "w16": W16,
            "ids": ids[b],
            "sbias": ib[b],
            "bproj": bp,
        })
    res = run_bass_kernel_spmd(nc, in_maps, core_ids=list(range(N_CORES)))
    out = np.empty((B, T, V), dtype=np.float32)
    for c in range(N_CORES):
        b, th = c // 2, c % 2
        shard = res.results[c]["out"]          # [V, TC]
        out[b, th * TC:(th + 1) * TC, :] = shard.T
    return out


if __name__ == "__main__":
    nc = build_bass()
    print("built ok")
